# revision 18
# baseline (speedup 1.0000x reference)
"""Trainium2 Bass kernel for the low-rank linear operator.

Math: the reference collapses algebraically. With y = linspace(-1,1,H),
x = linspace(-1,1,W), dx = 2/(W-1):

  Vy[b,i] = sum_{h,w} v[b,i,h,w] * y_h
  Vx[b,i] = sum_{h,w} v[b,i,h,w] * x_w
  inner[b,r] = dx * sum_i (Vy[b,i]*psi[r,i,0] + Vx[b,i]*psi[r,i,1])
  A[b,o] = sum_r inner[b,r]*phi[o,r,0];  Bc[b,o] = sum_r inner[b,r]*phi[o,r,1]
  u[b,o,h,w] = A[b,o]*y_h + Bc[b,o]*x_w

Sharding: data-parallel over batch, 2 batches per core, 8 cores, no
collectives. Per core: PE reduces over h via matmuls with a [y_h, 1]
stationary ([2,512] psum tiles per channel pair); ACT/DVE drain psum to
SBUF; one SBUF->SBUF DMA re-partitions to [128(2i+p), w]; two full-width
DVE ops finish the w-reduction (weights 1 / x_w interleaved); tiny PE
matmuls produce inner -> (A,B) and broadcast them into per-partition
scale/bias vectors; DVE/ACT/Pool generate u tiles as x_w*B + y_h*A; 1MB
batched DMAs both directions.
"""

import sys

try:
    import concourse.bass as bass  # noqa: F401
except ImportError:
    for _p in ("/opt/trn_rl_repo", "/root/.axon_site/_ro/trn_rl_repo"):
        if _p not in sys.path:
            sys.path.insert(0, _p)

import numpy as np

import concourse.bacc as bacc
import concourse.bass as bass
import concourse.mybir as mybir
import concourse.tile as tile
from concourse.bass_utils import run_bass_kernel_spmd

F32 = mybir.dt.float32
MULT = mybir.AluOpType.mult
ADD = mybir.AluOpType.add

B, CI, CO, R, H, W = 16, 64, 64, 64, 256, 256
N_CORES = 8
BPC = B // N_CORES  # batches per core

# generation-engine rotation
_GEN_ENGINES = ("act", "pool", "dve", "act", "pool", "act", "pool", "dve")


def build_nc():
    nc = bacc.Bacc("TRN2", target_bir_lowering=False, debug=False)

    v = nc.dram_tensor("v", [BPC, CI, H, W], F32, kind="ExternalInput")
    psi2 = nc.dram_tensor("psi2", [2 * CI, R], F32, kind="ExternalInput")
    phicat = nc.dram_tensor("phicat", [R, 2 * CO], F32, kind="ExternalInput")
    wt = nc.dram_tensor("wt", [2 * CI, W], F32, kind="ExternalInput")
    y2 = nc.dram_tensor("y2", [128, 4], F32, kind="ExternalInput")
    xrep = nc.dram_tensor("xrep", [128, W], F32, kind="ExternalInput")
    ybc = nc.dram_tensor("ybc", [1, 384], F32, kind="ExternalInput")
    ident1 = nc.dram_tensor("ident1", [1, 1], F32, kind="ExternalInput")
    u = nc.dram_tensor("u", [BPC, CO, H, W], F32, kind="ExternalOutput")

    IBLK = 8          # channels per input DMA
    NBLK = CI // IBLK
    OBLK = 4          # output channels per output DMA
    NOBLK = CO // OBLK

    with tile.TileContext(nc) as tc:
        with (
            tc.tile_pool(name="consts", bufs=1) as consts,
            tc.tile_pool(name="inp", bufs=4) as in_pool,
            tc.tile_pool(name="outp", bufs=4) as out_pool,
            tc.tile_pool(name="sA", bufs=2) as s_pool,
            tc.tile_pool(name="scr", bufs=3) as scratch,
            tc.tile_pool(name="bc", bufs=6) as bc_pool,
            tc.tile_pool(name="psumP", bufs=5, space="PSUM") as psum_p,
            tc.tile_pool(name="psumT", bufs=1, space="PSUM") as psum_t,
            tc.tile_pool(name="psumBC", bufs=2, space="PSUM") as psum_bc,
            tc.tile_pool(name="dram", bufs=2, space="DRAM") as dram_pool,
        ):
            sb_psi2 = consts.tile([2 * CI, R], F32)
            nc.sync.dma_start(sb_psi2[:], psi2[:])
            sb_phicat = consts.tile([R, 2 * CO], F32)
            nc.sync.dma_start(sb_phicat[:], phicat[:])
            sb_wt = consts.tile([2 * CI, W], F32)
            nc.sync.dma_start(sb_wt[:], wt[:])
            sb_y2 = consts.tile([128, 4], F32)
            nc.sync.dma_start(sb_y2[:], y2[:])
            sb_xrep = consts.tile([128, W], F32)
            nc.sync.dma_start(sb_xrep[:], xrep[:])
            sb_ybc = consts.tile([1, 384], F32)
            nc.sync.dma_start(sb_ybc[:], ybc[:])
            sb_id1 = consts.tile([1, 1], F32)
            nc.sync.dma_start(sb_id1[:], ident1[:])

            # Vy/Vx per (b): partition 2i = Vy[b,i], 2i+1 = Vx[b,i]
            gcat = consts.tile([2 * CI, BPC], F32)

            def phase_a(b):
                """Reduce v[b] -> gcat[:, b]."""
                dscr = dram_pool.tile([CI, 2, W], F32, tag="dscr")
                drain = 0
                for blk in range(NBLK):
                    i0 = blk * IBLK
                    t0 = in_pool.tile([128, IBLK, W], F32, tag="in")
                    nc.sync.dma_start(
                        t0[:], v[b, i0 : i0 + IBLK, 0:128, :].rearrange("i h w -> h i w")
                    )
                    t1 = in_pool.tile([128, IBLK, W], F32, tag="in")
                    nc.sync.dma_start(
                        t1[:], v[b, i0 : i0 + IBLK, 128:256, :].rearrange("i h w -> h i w")
                    )
                    pj = []
                    for j in range(IBLK // 2):
                        p = psum_p.tile([2, 2, W], F32, tag="P")
                        pj.append(p)
                        nc.tensor.matmul(
                            p[:], lhsT=sb_y2[:, 0:2], rhs=t0[:, 2 * j : 2 * j + 2, :],
                            start=True, stop=False,
                        )
                    for j in range(IBLK // 2):
                        nc.tensor.matmul(
                            pj[j][:], lhsT=sb_y2[:, 2:4], rhs=t1[:, 2 * j : 2 * j + 2, :],
                            start=False, stop=True,
                        )
                    s_blk = scratch.tile([2, IBLK, W], F32, tag="sblk")
                    for j in range(IBLK // 2):
                        dst = s_blk[:, 2 * j : 2 * j + 2, :]
                        if drain % 2 == 0:
                            nc.scalar.copy(dst, pj[j][:])
                        else:
                            nc.vector.tensor_copy(dst, pj[j][:])
                        drain += 1
                    nc.sync.dma_start(
                        dscr[i0 : i0 + IBLK, :, :].rearrange("i p w -> p i w"),
                        s_blk[:],
                    )
                # re-partition on readback: dscr[p, i, w] -> s2[2i+p, w]
                s2 = scratch.tile([2 * CI, W], F32, tag="s2")
                nc.sync.dma_start(s2[:], dscr[:].rearrange("i p w -> (i p) w"))
                sc2 = scratch.tile([2 * CI, W], F32, tag="sc2")
                nc.vector.tensor_tensor(out=sc2[:], in0=s2[:], in1=sb_wt[:], op=MULT)
                nc.vector.tensor_reduce(
                    out=gcat[:, b : b + 1], in_=sc2[:],
                    axis=mybir.AxisListType.X, op=ADD,
                )

            def tiny(b):
                """gcat[:, b] -> per-partition scale/bias SBUF tiles for batch b."""
                inner_ps = psum_t.tile([1, R], F32, tag="tiny")
                nc.tensor.matmul(
                    inner_ps[:], lhsT=gcat[:, b : b + 1], rhs=sb_psi2[:],
                    start=True, stop=True,
                )
                sb_inner = scratch.tile([1, R], F32, tag="ti1")
                nc.vector.tensor_copy(sb_inner[:], inner_ps[:])

                innert_ps = psum_t.tile([R, 1], F32, tag="tiny")
                nc.tensor.transpose(innert_ps[:], sb_inner[:], sb_id1[:])
                sb_innert = scratch.tile([R, 1], F32, tag="ti2")
                nc.vector.tensor_copy(sb_innert[:], innert_ps[:])

                ab_ps = psum_t.tile([1, 2 * CO], F32, tag="tiny")
                nc.tensor.matmul(
                    ab_ps[:], lhsT=sb_innert[:], rhs=sb_phicat[:],
                    start=True, stop=True,
                )
                sb_ab = scratch.tile([1, 2 * CO], F32, tag="ti3")
                nc.vector.tensor_copy(sb_ab[:], ab_ps[:])

                outs = []
                for k in range(3):  # bias_h0, bias_h1, scale
                    ps = psum_bc.tile([128, 2 * CO], F32, tag="bc")
                    nc.tensor.matmul(
                        ps[:],
                        lhsT=sb_ybc[0:1, 128 * k : 128 * (k + 1)],
                        rhs=sb_ab[:],
                        start=True,
                        stop=True,
                    )
                    sb = bc_pool.tile([128, 2 * CO], F32, tag="bcs")
                    nc.vector.tensor_copy(sb[:], ps[:])
                    outs.append(sb)
                return outs  # [bias_h0, bias_h1, scale]

            def phase_b(b, bias0, bias1, scale):
                eng = 0
                for oc in range(NOBLK):
                    ot = out_pool.tile([128, OBLK, 2, W], F32, tag="out")
                    for ol in range(OBLK):
                        o = oc * OBLK + ol
                        sc_ap = scale[:, 2 * o + 1 : 2 * o + 2]
                        for hb in range(2):
                            bias = (bias0 if hb == 0 else bias1)[:, 2 * o : 2 * o + 1]
                            dst = ot[:, ol, hb, :]
                            which = _GEN_ENGINES[eng % len(_GEN_ENGINES)]
                            eng += 1
                            if which == "dve":
                                nc.vector.tensor_scalar(
                                    out=dst, in0=sb_xrep[:], scalar1=sc_ap,
                                    scalar2=bias, op0=MULT, op1=ADD,
                                )
                            elif which == "pool":
                                nc.gpsimd.tensor_scalar(
                                    out=dst, in0=sb_xrep[:], scalar1=sc_ap,
                                    scalar2=bias, op0=MULT, op1=ADD,
                                )
                            else:
                                nc.scalar.activation(
                                    dst, sb_xrep[:],
                                    mybir.ActivationFunctionType.Identity,
                                    bias=bias, scale=sc_ap,
                                )
                    nc.sync.dma_start(
                        u[b, oc * OBLK : (oc + 1) * OBLK, :, :].rearrange(
                            "o (hb p) w -> p o hb w", p=128
                        ),
                        ot[:],
                    )

            phase_a(0)
            sb0 = tiny(0)
            phase_a(1)
            phase_b(0, *sb0)
            sb1 = tiny(1)
            phase_b(1, *sb1)

    nc.compile()
    return nc


def make_in_maps(v, psi, phi):
    y = np.linspace(-1.0, 1.0, H, dtype=np.float32)
    x = np.linspace(-1.0, 1.0, W, dtype=np.float32)
    dx = np.float32(2.0 / (W - 1))
    ones = np.ones(128, dtype=np.float32)
    psi2 = np.empty((2 * CI, R), np.float32)
    psi2[0::2, :] = psi[:, :, 0].T * dx
    psi2[1::2, :] = psi[:, :, 1].T * dx
    phicat = np.stack([phi[:, :, 0].T, phi[:, :, 1].T], axis=2).reshape(R, 2 * CO)
    wt = np.empty((2 * CI, W), np.float32)
    wt[0::2, :] = 1.0
    wt[1::2, :] = x
    shards = np.ascontiguousarray(v.reshape(N_CORES, BPC, CI, H, W))
    common = {
        "psi2": psi2,
        "phicat": np.ascontiguousarray(phicat),
        "wt": wt,
        "y2": np.stack([y[:128], ones, y[128:], ones], axis=1).astype(np.float32),
        "xrep": np.broadcast_to(x, (128, W)).copy(),
        "ybc": np.concatenate([y[:128], y[128:], ones])[None, :].astype(np.float32),
        "ident1": np.ones((1, 1), dtype=np.float32),
    }
    return [{"v": shards[i], **common} for i in range(N_CORES)]


_NC_CACHE = None


def kernel(v, psi, phi):
    global _NC_CACHE
    if _NC_CACHE is None:
        _NC_CACHE = build_nc()
    nc = _NC_CACHE
    in_maps = make_in_maps(
        np.ascontiguousarray(v, dtype=np.float32),
        np.asarray(psi, dtype=np.float32),
        np.asarray(phi, dtype=np.float32),
    )
    res = run_bass_kernel_spmd(nc, in_maps, core_ids=list(range(N_CORES)))
    return np.concatenate([r["u"] for r in res.results], axis=0)


if __name__ == "__main__":
    build_nc()
    print("build ok")
